# revision 20
# baseline (speedup 1.0000x reference)
"""Trainium2 Bass kernel for nn_ClassificationHead (MetaOptNet-Ridge head).

Per task t (256 total): K = S_t S_t^T + 50 I  (25x25);  X = 2 K^{-1} Y_t;
W = S_t^T X (640x5);  logits_t = scale * Q_t W  (300x5).

The end-to-end metric is dominated by host->device transfer of q
(256x300x640), so q is quantized on host to int8 with one scale per
query row; the scales never reach the device -- logits come back
unscaled and the host multiplies them in during gather. s is likewise
int8 per-row (dequantized to f16 on device); identity/mask constants
are synthesized on device, so the wire carries only q, s, ssc, y16.

Device (8 NeuronCores, pure task parallelism, 32 tasks/core):
  - tasks grouped 5-at-a-time into 125x125 block-diagonal systems
  - K^{-1} via Newton-Schulz: M1 = 2aI - a^2 K closed form, 2 bf16 Newton
    iterations, then X via 2 fp32 iterative-refinement steps
  - q int8 -> f16 cast (DVE), Q^T via PE transposes, logits^T = W^T Q^T
    on the TensorEngine (f16); device emits logits^T [5, 300] f16
  - Q_INT8=False fallback: q arrives f16 and Q^T is built by a single
    xbar DMA-transpose instead (no casts, no PE transposes)
"""

import numpy as np
import ml_dtypes

import concourse.bass as bass
import concourse.tile as tile
from concourse import bacc, mybir
from concourse.bass import MemorySpace, ds
from concourse.bass_utils import run_bass_kernel_spmd

F32 = mybir.dt.float32
F16 = mybir.dt.float16
BF16 = mybir.dt.bfloat16
I8 = mybir.dt.int8
NPBF16 = ml_dtypes.bfloat16

# problem shapes (hardcoded per contract)
T, NQ, NS, D, W = 256, 300, 25, 640, 5
CORES = 8
TPC = T // CORES          # 32 tasks per core
GT = 5                    # tasks per block-diag group
G = (TPC + GT - 1) // GT  # 7 groups (last group padded with 3 dummy tasks)
PT = G * GT               # 35 padded tasks per core
GP = GT * NS              # 125 partitions per group
DC = D // 128             # 5 contraction chunks
TQ = TPC * NQ             # 9600 query rows per core
QP = 100                  # query rows per transpose block (3 x 100 = 300)

ALPHA = 1.4e-3            # Newton-Schulz seed: K eigs in ~[433, 1016]
LAMBDA = 50.0

Q_INT8 = True             # int8 q + host-side output scaling


def build_nc(q_int8=Q_INT8):
    nc = bacc.Bacc("TRN2", target_bir_lowering=False, debug=False,
                   num_devices=CORES)

    if q_int8:
        q = nc.dram_tensor("q", [TPC, NQ, D], I8, kind="ExternalInput")
    else:
        q = nc.dram_tensor("q", [TQ, D], F16, kind="ExternalInput")
    # s/ssc/y16 partition-major so each loads in ONE DMA (the ~1.7us
    # fixed cost per DMA dominates their transfer time otherwise)
    s = nc.dram_tensor("s", [GP, G, D], I8, kind="ExternalInput")
    ssc = nc.dram_tensor("ssc", [GP, G, 1], F32, kind="ExternalInput")
    y16 = nc.dram_tensor("y16", [GP, G, NS], BF16, kind="ExternalInput")
    o = nc.dram_tensor("o", [TPC, W, NQ], F16, kind="ExternalOutput")

    with tile.TileContext(nc) as tc:
        with (
            tc.tile_pool(name="consts", bufs=1) as consts,
            tc.tile_pool(name="grp", bufs=2) as grp,
            tc.tile_pool(name="slv", bufs=2) as slv,
            tc.tile_pool(name="qp", bufs=3) as qp,
            tc.tile_pool(name="qtp", bufs=3) as qtp,
            tc.tile_pool(name="ps_sv", bufs=3, space=MemorySpace.PSUM) as ps_sv,
            tc.tile_pool(name="ps_qt", bufs=3, space=MemorySpace.PSUM) as ps_qt,
            tc.tile_pool(name="ps_lg", bufs=2, space=MemorySpace.PSUM) as ps_lg,
        ):
            if not q_int8:
                # all of Q^T in one xbar DMA-transpose:
                # qt[p, c, t*300+r] = q[t*300+r, 128c+p]
                qt_all = consts.tile([128, DC, TQ], F16)
                nc.sync.dma_start(out=qt_all, in_=q[:, :], transpose=True)

            # constants are synthesized on device (no transfer):
            # identity via affine_select, block-diag mask via B^T B outer
            ones16 = consts.tile([128, 128], F16)
            nc.vector.memset(ones16, 1.0)
            c_id16 = consts.tile([128, 128], F16)
            nc.gpsimd.affine_select(
                out=c_id16, in_=ones16, pattern=[[-1, 128]], base=0,
                channel_multiplier=1, compare_op=mybir.AluOpType.is_equal,
                fill=0.0)
            bt0 = consts.tile([GT, GP], F16)
            nc.gpsimd.affine_select(
                out=bt0, in_=ones16[:GT, :GP], pattern=[[1, GP]], base=0,
                channel_multiplier=-NS, compare_op=mybir.AluOpType.is_ge,
                fill=0.0)
            bt = consts.tile([GT, GP], F16)
            nc.gpsimd.affine_select(
                out=bt, in_=bt0, pattern=[[-1, GP]], base=NS - 1,
                channel_multiplier=NS, compare_op=mybir.AluOpType.is_ge,
                fill=0.0)
            mkp = ps_sv.tile([GP, GP], F32, tag="sv")
            nc.tensor.matmul(mkp, bt, bt)
            c_mask = consts.tile([GP, GP], F32)
            nc.vector.tensor_copy(out=c_mask, in_=mkp)
            c_twoI = consts.tile([GP, GP], F32)
            nc.scalar.mul(out=c_twoI, in_=c_id16[:GP, :GP], mul=2.0)
            c_t2aI = consts.tile([GP, GP], F32)
            nc.scalar.mul(out=c_t2aI, in_=c_id16[:GP, :GP], mul=2.0 * ALPHA)
            c_fifI = consts.tile([GP, GP], F32)
            nc.scalar.mul(out=c_fifI, in_=c_id16[:GP, :GP], mul=LAMBDA)

            # bulk-load all groups' s / ssc / y16 in one DMA each
            s_all = consts.tile([GP, G, D], I8)
            nc.scalar.dma_start(out=s_all, in_=s[:, :, :])
            ssc_all = consts.tile([GP, G, 1], F32)
            nc.scalar.dma_start(out=ssc_all, in_=ssc[:, :, :])
            y_all = consts.tile([GP, G, NS], BF16)
            nc.scalar.dma_start(out=y_all, in_=y16[:, :, :])

            # ---- group solves: K -> M ~ K^{-1} -> X -> W (all 7 groups) ----
            w5s = []
            for g in range(G):
                s5 = grp.tile([GP, D], F16, tag="s5")
                nc.vector.tensor_scalar_mul(s5, s_all[:, g, :],
                                            ssc_all[:, g, :])
                y16t = y_all[:, g, :]
                y32t = grp.tile([GP, NS], F32, tag="y32")
                nc.vector.tensor_copy(out=y32t, in_=y16t)

                # S^T chunks [128, 125] x 5 via PE transpose
                st5 = grp.tile([128, DC, GP], F16, tag="st5")
                for c in range(DC):
                    tp = ps_sv.tile([128, GP], F16, tag="sv")
                    nc.tensor.transpose(tp, s5[:, ds(128 * c, 128)],
                                        c_id16[:GP, :GP])
                    nc.scalar.copy(out=st5[:, c, :], in_=tp)

                # cross-Gram, then mask to block-diag + 50 I
                gram = ps_sv.tile([GP, GP], F32, tag="sv")
                for c in range(DC):
                    nc.tensor.matmul(gram, st5[:, c, :], st5[:, c, :],
                                     start=(c == 0), stop=(c == DC - 1))
                k32 = slv.tile([GP, GP], F32, tag="k32")
                nc.vector.tensor_mul(k32, gram, c_mask)
                nc.vector.tensor_add(k32, k32, c_fifI)
                k16 = slv.tile([GP, GP], BF16, tag="k16")
                nc.vector.tensor_copy(out=k16, in_=k32)

                # M1 = 2aI - a^2 K, then 2 bf16 Newton-Schulz iterations
                m16 = slv.tile([GP, GP], BF16, tag="m16")
                nc.scalar.mul(out=m16, in_=k32, mul=-ALPHA * ALPHA)
                nc.vector.tensor_add(m16, m16, c_t2aI)
                for _ in range(2):
                    pp = ps_sv.tile([GP, GP], F32, tag="sv")
                    nc.tensor.matmul(pp, k16, m16)
                    r16 = slv.tile([GP, GP], BF16, tag="r16")
                    nc.vector.tensor_sub(r16, c_twoI, pp)
                    mp = ps_sv.tile([GP, GP], F32, tag="sv")
                    nc.tensor.matmul(mp, m16, r16)
                    m16 = slv.tile([GP, GP], BF16, tag="m16")
                    nc.vector.tensor_copy(out=m16, in_=mp)

                # X0 = M Y, then 2 fp32 iterative-refinement steps
                xp = ps_sv.tile([GP, NS], F32, tag="sv")
                nc.tensor.matmul(xp, m16, y16t)
                xf = slv.tile([GP, NS], F32, tag="xf")
                nc.vector.tensor_copy(out=xf, in_=xp)
                for _ in range(2):
                    rp = ps_sv.tile([GP, NS], F32, tag="sv")
                    nc.tensor.matmul(rp, k32, xf)
                    r16s = slv.tile([GP, NS], BF16, tag="r16s")
                    nc.vector.tensor_sub(r16s, y32t, rp)
                    dxp = ps_sv.tile([GP, NS], F32, tag="sv")
                    nc.tensor.matmul(dxp, m16, r16s)
                    nc.vector.tensor_add(xf, xf, dxp)
                xf16 = slv.tile([GP, NS], F16, tag="xf16")
                nc.vector.tensor_copy(out=xf16, in_=xf)

                # W5[:, c, 5j:5j+5] = (S_t^T X_t) rows for chunk c, task j
                w5 = consts.tile([128, DC, NS], F16)
                for c in range(DC):
                    wp = ps_sv.tile([128, NS], F32, tag="sv")
                    nc.tensor.matmul(wp, s5[:, ds(128 * c, 128)], xf16)
                    nc.scalar.copy(out=w5[:, c, :], in_=wp)
                w5s.append(w5)

            # ---- per-task logits: logits^T = W^T Q^T ----
            lgg = consts.tile([W, TPC, NQ], F16)
            for g in range(G):
                jn = min(GT, TPC - g * GT)
                for j in range(jn):
                    t = g * GT + j
                    if q_int8:
                        qsb = qp.tile([QP, 3, D], I8, tag="qsb")
                        nc.sync.dma_start(
                            out=qsb,
                            in_=q[t].rearrange("(c p) d -> p c d", p=QP))
                        qc = qp.tile([QP, 3, D], F16, tag="qc")
                        nc.vector.tensor_copy(out=qc, in_=qsb)
                        qt_sb = qtp.tile([128, DC, NQ], F16, tag="qt")
                        for c in range(DC):
                            qtps = ps_qt.tile([128, NQ], F16, tag="qt")
                            for cc in range(3):
                                nc.tensor.transpose(
                                    qtps[:, ds(QP * cc, QP)],
                                    qc[:, cc, ds(128 * c, 128)],
                                    c_id16[:QP, :QP])
                            if (t * DC + c) % 2 == 0:
                                nc.scalar.copy(out=qt_sb[:, c, :], in_=qtps)
                            else:
                                nc.vector.tensor_copy(out=qt_sb[:, c, :],
                                                      in_=qtps)
                        qt_c = qt_sb
                        qoff = 0
                    else:
                        qt_c = qt_all
                        qoff = NQ * t
                    lgp = ps_lg.tile([W, NQ], F32, tag="lg")
                    for c in range(DC):
                        nc.tensor.matmul(lgp, w5s[g][:, c, ds(W * j, W)],
                                         qt_c[:, c, ds(qoff, NQ)],
                                         start=(c == 0), stop=(c == DC - 1))
                    nc.scalar.copy(out=lgg[:, t, :], in_=lgp)
            nc.scalar.dma_start(out=o.rearrange("t w q -> w t q"), in_=lgg)

    nc.compile()
    return nc


_JAX = {}


def _jax_cpu():
    if "cpu" not in _JAX:
        try:
            import jax
            _JAX["jax"] = jax
            _JAX["cpu"] = jax.local_devices(backend="cpu")[0]
        except Exception:
            _JAX["cpu"] = None
    return _JAX.get("cpu")


def _quant_int8(x):
    """Per-row symmetric int8: returns (x_int8 (..,R,D), scales f32 (..,R,1))."""
    cpu = _jax_cpu()
    if cpu is not None:
        jax, jnp = _JAX["jax"], __import__("jax.numpy", fromlist=["numpy"])
        if "quant" not in _JAX:
            def _f(q):
                sc = jnp.max(jnp.abs(q), axis=-1, keepdims=True) / 127.0
                qi = jnp.clip(jnp.rint(q / sc), -127, 127).astype(jnp.int8)
                return qi, sc
            _JAX["quant"] = jax.jit(_f, device=cpu)
        qi, sc = _JAX["quant"](x)
        return np.asarray(qi), np.asarray(sc)
    sc = np.abs(x).max(axis=-1, keepdims=True) / 127.0
    qi = np.clip(np.rint(x / sc), -127, 127).astype(np.int8)
    return qi, sc


def _fast_f16(x):
    cpu = _jax_cpu()
    if cpu is not None:
        jax = _JAX["jax"]
        return np.asarray(jax.device_put(x, cpu).astype("float16"))
    return x.astype(np.float16)


def _host_prep(query, support, scale, support_labels):
    """Build the 8 per-core input maps + host-side logit scales."""
    query = np.asarray(query)
    support = np.asarray(support)
    scale_v = float(np.asarray(scale).reshape(-1)[0])
    labels = np.asarray(support_labels).astype(np.int64)

    if Q_INT8:
        qi, sc = _quant_int8(query)
        qf = qi.reshape(CORES, TPC, NQ, D)
    else:
        qf = _fast_f16(query).reshape(CORES, TQ, D)
        sc = None

    si, ssc = _quant_int8(support)
    s_pad = np.zeros((CORES, PT, NS, D), dtype=np.int8)
    s_pad[:, :TPC] = si.reshape(CORES, TPC, NS, D)
    # -> partition-major [GP = gt*NS + ns, G] for single-DMA loads
    s_pad = np.ascontiguousarray(
        s_pad.reshape(CORES, G, GT, NS, D).transpose(0, 2, 3, 1, 4)
    ).reshape(CORES, GP, G, D)
    ssc_pad = np.zeros((CORES, PT, NS, 1), dtype=np.float32)
    ssc_pad[:, :TPC] = ssc.reshape(CORES, TPC, NS, 1)
    ssc_pad = np.ascontiguousarray(
        ssc_pad.reshape(CORES, G, GT, NS, 1).transpose(0, 2, 3, 1, 4)
    ).reshape(CORES, GP, G, 1)

    lab_pad = np.zeros((CORES, PT, NS), dtype=np.int64)
    lab_pad[:, :TPC] = labels.reshape(CORES, TPC, NS)
    oh = (lab_pad[..., None] == np.arange(W)).astype(np.float32)
    oh = (oh * (2.0 * scale_v)).reshape(CORES, G, GT, NS, W)
    y = np.zeros((CORES, G, GT, NS, GT, W), dtype=np.float32)
    for j in range(GT):
        y[:, :, j, :, j, :] = oh[:, :, j]
    y16 = np.ascontiguousarray(
        y.reshape(CORES, G, GP, NS).transpose(0, 2, 1, 3)).astype(NPBF16)

    in_maps = [{"q": qf[core], "s": s_pad[core], "ssc": ssc_pad[core],
                "y16": y16[core]} for core in range(CORES)]
    return in_maps, sc


def _host_inputs(query, support, scale, support_labels):
    return _host_prep(query, support, scale, support_labels)[0]


_NC_CACHE = {}


def _get_nc():
    if "nc" not in _NC_CACHE:
        _NC_CACHE["nc"] = build_nc()
    return _NC_CACHE["nc"]


def kernel(query, support, scale, support_labels, n_way=5, n_shot=5, **_):
    assert int(n_way) == W and np.asarray(query).shape == (T, NQ, D)
    nc = _get_nc()
    in_maps, sc = _host_prep(query, support, scale, support_labels)
    res = run_bass_kernel_spmd(nc, in_maps, core_ids=list(range(CORES)))
    # gather: per-core [32, 5, 300] f16 -> [256, 300, 5] f32 (+ row scales)
    full = np.concatenate([r["o"] for r in res.results], axis=0)
    out = full.astype(np.float32)
    if sc is not None:
        out *= sc.transpose(0, 2, 1)  # (T, 1, NQ) broadcast over W
    return np.ascontiguousarray(out.transpose(0, 2, 1))
